# revision 1
# baseline (speedup 1.0000x reference)
"""Dynamic-expert-conv kernel for Trainium2 (8 NeuronCores, SPMD data-parallel).

Problem: per-sample expert-mixed 3x3 conv:
    w[b] = sum_e attention[b,e] * weights[e]     # [O, C, 3, 3]
    out[b] = conv2d(input[b], w[b], pad=1) + bias_mix[b][:, None, None]

Strategy (per core, 4 samples):
  - Expert weight bank resident in SBUF as bf16 (halves HBM+SBUF cost).
  - Per-sample combined weights built on VectorE in fp32 (fused
    scalar_tensor_tensor MACs, no per-step bf16 rounding), then cast to
    bf16 per (o-chunk, c-chunk) quarter, still on VectorE, overlapped
    with TensorE conv of the previous sample.
  - Conv as accumulating bf16 PE matmuls with the STATIONARY operand
    reused across y-tiles: for each (sample, o-chunk), loop (c-chunk,
    offset) OUTER and y-tiles INNER over a group of PSUM banks, so one
    weight load (fast-weight-load eligible: bf16, 128 cols) feeds 4x448
    or 3x448 moving columns instead of 1x448.
  - ScalarE evacuates PSUM -> SBUF fp32 with the mixed bias fused in.
  - Input is bf16, streamed per whole sample (host pre-padded to 58x58;
    input rounding ~2^-9 relative, well inside the 2e-2 gate).
"""
import numpy as np

import concourse.bass as bass
import concourse.tile as tile
from concourse import bacc, mybir
from concourse.bass import ts
from concourse.bass_utils import run_bass_kernel_spmd
from contextlib import ExitStack

F32 = mybir.dt.float32
BF16 = mybir.dt.bfloat16

B, C, O, H, W, KK, E = 32, 256, 256, 56, 56, 3, 8
N_CORES = 8
B_LOC = B // N_CORES          # 4 samples per core
PW = H + 2                    # 58 padded
CCH = C // 128                # 2
OCH = O // 128                # 2
YT = 8                        # output rows per tile
NT = H // YT                  # 7 y-tiles
DYX = KK * KK                 # 9
QH = DYX * 128                # 1152 combined-weight cols per (c-chunk, o-chunk)

# y-tile groups sharing one stationary-load sweep (4+3 PSUM banks)
GROUPS = [(0, 4), (4, 3)]


def _dedup_ldweights(nc):
    """Drop redundant PE weight reloads.

    Tile lowering splits every bf16 matmul into Ldweights+Matmult; the HW
    pays a serial ~54ns per LDWEIGHTS and does not skip reloads of the
    already-loaded stationary. conv_group orders matmuls so consecutive
    ones share the stationary, so any Ldweights whose access pattern
    matches the previous Ldweights in the same block's PE stream (with no
    waits/updates of its own) is a no-op and can be deleted: the weights
    are still in the array, and its (empty) sync carries nothing."""
    removed = 0
    for blk in nc.m.functions[0].blocks:
        last_key = None
        keep = []
        for inst in blk.instructions:
            if isinstance(inst, mybir.InstLdweights):
                si = inst.sync_info
                clean = si is None or (not si.on_wait and not si.on_update)
                key = (str(inst.ins[0]), str(inst.tile_position),
                       str(inst.perf_mode), str(inst.is_transpose))
                if clean and key == last_key:
                    removed += 1
                    continue
                last_key = key
            keep.append(inst)
        if removed:
            blk.instructions = keep
    return removed


def build(iters: int = 1):
    nc = bacc.Bacc("TRN2", target_bir_lowering=False, debug=False,
                   num_devices=N_CORES)
    x = nc.dram_tensor("x", [B_LOC, 128, CCH, PW, PW], BF16,
                       kind="ExternalInput").ap()
    bank = nc.dram_tensor("bank", [E, 128, CCH, OCH, QH], BF16,
                          kind="ExternalInput").ap()
    att = nc.dram_tensor("att", [128, B_LOC * E], F32,
                         kind="ExternalInput").ap()
    bias_t = nc.dram_tensor("bias_t", [128, OCH, E], F32,
                            kind="ExternalInput").ap()
    out = nc.dram_tensor("out", [B_LOC, 128, OCH, H, W], F32,
                         kind="ExternalOutput").ap()

    with ExitStack() as ctx:
        tc = ctx.enter_context(tile.TileContext(nc))
        const = ctx.enter_context(tc.tile_pool(name="const", bufs=1))
        bankp = ctx.enter_context(tc.tile_pool(name="bankp", bufs=1))
        cfp = ctx.enter_context(tc.tile_pool(name="cfp", bufs=2))
        combp = ctx.enter_context(tc.tile_pool(name="combp", bufs=2))
        sampp = ctx.enter_context(tc.tile_pool(name="sampp", bufs=2))
        stagep = ctx.enter_context(tc.tile_pool(name="stagep", bufs=6))
        psump = ctx.enter_context(tc.tile_pool(name="psump", bufs=8,
                                               space="PSUM"))

        att_sb = const.tile([128, B_LOC * E], F32)
        nc.sync.dma_start(att_sb[:], att[:])
        bias_sb = const.tile([128, OCH, E], F32)
        nc.sync.dma_start(bias_sb[:], bias_t[:])

        bank_sb = bankp.tile([128, E, CCH, OCH, QH], BF16)
        # Stream the bank in (o-chunk, c-chunk) quarters so combining and
        # conv matmuls for the first quarter start after ~1/4 of the bank
        # has arrived from HBM; alternate experts across the sync and
        # gpsimd DMA rings (gpsimd is idle until the first output DMA)
        # so the first quarter lands ~2x sooner.
        for j in range(OCH):
            for k in range(CCH):
                for e in range(E):
                    ring = nc.sync if e % 2 == 0 else nc.gpsimd
                    ring.dma_start(bank_sb[:, e, k, j, :],
                                   bank[e][:, k, j, :])

        bias_comb = const.tile([128, B_LOC, OCH], F32)
        bias_junk = const.tile([128, E], F32)

        def combine_bias():
            # bias_comb[p, b, j] = sum_e bias_t[p, j, e] * att[p, b*E+e]
            for b in range(B_LOC):
                for j in range(OCH):
                    nc.vector.scalar_tensor_tensor(
                        bias_junk[:], bias_sb[:, j, :], 1.0,
                        att_sb[:, b * E:(b + 1) * E],
                        op0=mybir.AluOpType.mult, op1=mybir.AluOpType.mult,
                        accum_out=bias_comb[:, b, j:j + 1])

        def combine_quarter(b, cf, cb, j, k, lo=0, hi=QH):
            """cf[p,k,j,lo:hi] = sum_e att[b,e]*bank[p,e,k,j,lo:hi] in
            fp32 on VectorE, then cast the range to bf16 comb cb."""
            dst = cf[:, k, j, lo:hi]
            srcs = [bank_sb[:, e, k, j, lo:hi] for e in range(E)]
            nc.vector.tensor_scalar_mul(dst, srcs[0],
                                        att_sb[:, b * E:b * E + 1])
            for e in range(1, E):
                nc.vector.scalar_tensor_tensor(
                    dst, srcs[e], att_sb[:, b * E + e:b * E + e + 1], dst,
                    op0=mybir.AluOpType.mult, op1=mybir.AluOpType.add)
            nc.vector.tensor_scalar_mul(cb[:, k, j, lo:hi], dst, 1.0)

        def combine_sample(b, cf, cb, with_bias=False):
            for j in range(OCH):
                for k in range(CCH):
                    # Sample 0, first o-chunk: split the combine into two
                    # half-quarter chains so the PE's first matmuls start
                    # after ~half the chain instead of all of it.
                    if with_bias and j == 0:
                        for lo, hi in ((0, 512), (512, QH)):
                            combine_quarter(b, cf, cb, j, k, lo, hi)
                    else:
                        combine_quarter(b, cf, cb, j, k)
                if with_bias and j == 0:
                    combine_bias()

        def load_samp(b):
            samp = sampp.tile([128, CCH, PW, PW], BF16, name="samp")
            nc.scalar.dma_start(samp[:], x[b][:])
            return samp

        def conv_group(b, j, t0, tn, comb, samp):
            """One stationary-load sweep over y-tiles [t0, t0+tn):
            (k,d) outer so consecutive matmuls share one combined-weight
            tile (_dedup_ldweights drops their redundant reloads) and
            stream tn x 448 moving columns per load; y-tiles accumulate
            in parallel PSUM banks."""
            psums = [psump.tile([128, YT, W], F32, name="psum")
                     for _ in range(tn)]
            for k in range(CCH):
                for d in range(DYX):
                    dy, dx = d // KK, d % KK
                    lhsT = comb[:, k, j, d * 128:(d + 1) * 128]
                    first = (k == 0 and d == 0)
                    last = (k == CCH - 1 and d == DYX - 1)
                    for i in range(tn):
                        r0 = (t0 + i) * YT + dy
                        rhs = samp[:, k, r0:r0 + YT, dx:dx + W]
                        nc.tensor.matmul(psums[i][:], lhsT, rhs,
                                         start=first, stop=last)
            for i in range(tn):
                stage = stagep.tile([128, YT, W], F32, name="stage")
                nc.scalar.activation(
                    stage[:], psums[i][:],
                    mybir.ActivationFunctionType.Identity,
                    bias=bias_comb[:, b, j:j + 1], scale=1.0)
                nc.gpsimd.dma_start(out[b][:, j:j + 1, ts(t0 + i, YT), :],
                                    stage[:])

        def body():
            samps = {0: load_samp(0)}
            cfs = {0: cfp.tile([128, CCH, OCH, QH], F32, name="cf")}
            combs = {0: combp.tile([128, CCH, OCH, QH], BF16, name="comb")}
            combine_sample(0, cfs[0], combs[0], with_bias=True)
            for b in range(B_LOC):
                if b + 1 < B_LOC:
                    samps[b + 1] = load_samp(b + 1)
                    cfs[b + 1] = cfp.tile([128, CCH, OCH, QH], F32,
                                          name="cf")
                    combs[b + 1] = combp.tile([128, CCH, OCH, QH], BF16,
                                              name="comb")
                    combine_sample(b + 1, cfs[b + 1], combs[b + 1])
                for j in range(OCH):
                    for t0, tn in GROUPS:
                        conv_group(b, j, t0, tn, combs[b], samps[b])

        if iters == 1:
            body()
        else:
            # On-device repeat loop — used only for slope-based HW timing.
            with tc.For_i(0, iters, 1, hint_engines=(mybir.EngineType.PE,)):
                body()

    _dedup_ldweights(nc)
    nc.compile()
    return nc


def prep_inputs(input, attention, weights, bias):
    """Host-side shard + layout prep. Returns per-core in_maps."""
    import ml_dtypes
    input = np.asarray(input, dtype=np.float32)
    attention = np.asarray(attention, dtype=np.float32)
    weights = np.asarray(weights, dtype=np.float32)
    bias = np.asarray(bias, dtype=np.float32)

    xp = np.zeros((B, CCH, 128, PW, PW), ml_dtypes.bfloat16)
    xp[:, :, :, 1:H + 1, 1:W + 1] = input.reshape(B, CCH, 128, H, W)
    xp = np.ascontiguousarray(xp.transpose(0, 2, 1, 3, 4))  # [B,128,CCH,PW,PW]

    # weights [E, O, C, ky, kx] -> bank[e, p(c_lo), c_chunk, o_chunk, (d,o_lo)]
    wt = weights.transpose(0, 2, 3, 4, 1)                    # [E, C, ky, kx, O]
    wt = wt.reshape(E, CCH, 128, DYX, OCH, 128)              # [E,k,p,d,j,o]
    bank = np.ascontiguousarray(wt.transpose(0, 2, 1, 4, 3, 5)
                                ).reshape(E, 128, CCH, OCH, QH)
    bank = bank.astype(ml_dtypes.bfloat16)

    bias_t = np.ascontiguousarray(
        bias.T.reshape(OCH, 128, E).transpose(1, 0, 2))      # [128, OCH, E]

    in_maps = []
    for m in range(N_CORES):
        sl = slice(m * B_LOC, (m + 1) * B_LOC)
        att_m = np.ascontiguousarray(
            np.broadcast_to(attention[sl].reshape(1, B_LOC * E),
                            (128, B_LOC * E)))
        in_maps.append({
            "x": np.ascontiguousarray(xp[sl]),
            "bank": bank,
            "att": att_m,
            "bias_t": bias_t,
        })
    return in_maps


def gather_output(results):
    """Per-core [B_LOC, 128, OCH, H, W] -> full [B, O, H, W]."""
    outs = []
    for m in range(N_CORES):
        o = results[m]["out"]  # [B_LOC, 128, OCH, H, W]
        outs.append(o.transpose(0, 2, 1, 3, 4).reshape(B_LOC, O, H, W))
    return np.concatenate(outs, axis=0)


_NC_CACHE = {}


def _get_nc():
    if "nc" not in _NC_CACHE:
        _NC_CACHE["nc"] = build()
    return _NC_CACHE["nc"]


def kernel(input, attention, weights, bias):
    nc = _get_nc()
    in_maps = prep_inputs(input, attention, weights, bias)
    res = run_bass_kernel_spmd(nc, in_maps, list(range(N_CORES)))
    return gather_output(res.results)

